# revision 15
# baseline (speedup 1.0000x reference)
"""DensityDenoiser edge-update kernel for 8x TRN2 NeuronCores.

Reference computation (per edge e with endpoints s=ei[0,e], d=ei[1,e]):
    x  = concat(node[s], node[d], ef[e])          # [320]
    h  = relu(x @ W1 + b1) @ W2 + b2              # [128]
    hn = (h - mean(h)) * rsqrt(var(h)+eps) * g + b
    out[e] = ef[e] + hn

Sharding: edge-parallel across 8 cores; node table + weights replicated.

Device pipeline per 512-edge block (4 edge-slots per partition):
  gather src/dst node rows (bf16) via indirect DMA -> PE transpose to
  channel-major -> 3 accumulating bf16 matmuls (W1 chunks) -> ACT relu+b1
  -> per-slot matmul with activations stationary (W2) giving edge-major h
  in PSUM + a row-sum matmul (W2@1/128) giving mean directly -> ACT square
  with accumulate for E[h^2] -> DVE normalize + residual -> f32 store.
"""

import numpy as np
import ml_dtypes

import concourse.bass as bass
import concourse.bacc as bacc
import concourse.mybir as mybir
import concourse.tile as tile
from concourse.bass_utils import run_bass_kernel_spmd

F32 = mybir.dt.float32
BF16 = mybir.dt.bfloat16
I32 = mybir.dt.int32




N_CORES = 8
N_NODES = 50000
H = 128      # hidden/edge channels
NID = 96     # node invariant dim
IN_DIM = H + 2 * NID
LN_EPS = 1e-5

K_BLK = 4                 # edge slots per partition per compute block
BLK = 128 * K_BLK         # 512 edges per compute block
BLKS_PER_SB = 4
K_SB = K_BLK * BLKS_PER_SB  # 16 slots per partition per superblock
SB = 128 * K_SB           # 2048 edges per DMA superblock

# How the node-table gather's index AP is laid out.  'multi' ([128, K_BLK]
# offsets) matches CoreSim but NOT the hardware SWDGE ucode; 'per128' is the
# proven one-index-per-partition pattern; 'flat' keeps all of a block's
# indices in one partition.
GATHER_MODE = "per128"


def _build_program(n_superblocks: int, general: bool):
    """Build the single-core SPMD Bass program."""
    nc = bacc.Bacc()

    e_core = n_superblocks * SB
    tbl = nc.declare_dram_parameter("tbl", [N_NODES, NID], BF16, isOutput=False)
    ef_d = nc.declare_dram_parameter("ef", [e_core, H], F32, isOutput=False)
    idx_shape = [n_superblocks, 1, SB] if GATHER_MODE == "flat" else [n_superblocks, 128, K_SB]
    si_d = nc.declare_dram_parameter("src_idx", idx_shape, I32, isOutput=False)
    di_d = nc.declare_dram_parameter("dst_idx", idx_shape, I32, isOutput=False)
    w1a_d = nc.declare_dram_parameter("w1a", [NID, H], BF16, isOutput=False)
    w1b_d = nc.declare_dram_parameter("w1b", [NID, H], BF16, isOutput=False)
    w1c_d = nc.declare_dram_parameter("w1c", [H, H], BF16, isOutput=False)
    w2_d = nc.declare_dram_parameter("w2", [H, H], BF16, isOutput=False)
    w2s_d = nc.declare_dram_parameter("w2sum", [H, 1], BF16, isOutput=False)  # W2@1/H
    b1_d = nc.declare_dram_parameter("b1", [H, 1], F32, isOutput=False)
    if general:
        b2_d = nc.declare_dram_parameter("b2row", [1, H], BF16, isOutput=False)
        mb2_d = nc.declare_dram_parameter("mb2", [1, 1], BF16, isOutput=False)
        gam_d = nc.declare_dram_parameter("gam4", [128, K_BLK, H], F32, isOutput=False)
        bet_d = nc.declare_dram_parameter("bet4", [128, K_BLK, H], F32, isOutput=False)
    out_d = nc.declare_dram_parameter("out", [e_core, H], F32, isOutput=True)

    with tile.TileContext(nc) as tc:
        with (
            tc.tile_pool(name="singles", bufs=1) as singles,
            tc.tile_pool(name="sbp", bufs=2) as sbp,
            tc.tile_pool(name="blk", bufs=3) as blk,
            tc.tile_pool(name="ps_tr", bufs=4, space="PSUM") as ps_tr,
            tc.tile_pool(name="ps_h1", bufs=2, space="PSUM") as ps_h1,
            tc.tile_pool(name="ps_h", bufs=2, space="PSUM") as ps_h,
        ):
            # ---- constants / weights (loaded once) ----
            ident = singles.tile([128, 128], BF16)
            from concourse.masks import make_identity
            make_identity(nc, ident[:])

            w1a = singles.tile([NID, H], BF16)
            w1b = singles.tile([NID, H], BF16)
            w1c = singles.tile([H, H], BF16)
            w2 = singles.tile([H, H], BF16)
            w2s = singles.tile([H, 1], BF16)
            b1 = singles.tile([H, 1], F32)
            eps_t = singles.tile([128, 1], F32)
            nc.sync.dma_start(out=w1a[:], in_=w1a_d[:, :])
            nc.sync.dma_start(out=w1b[:], in_=w1b_d[:, :])
            nc.sync.dma_start(out=w1c[:], in_=w1c_d[:, :])
            nc.sync.dma_start(out=w2[:], in_=w2_d[:, :])
            nc.sync.dma_start(out=w2s[:], in_=w2s_d[:, :])
            nc.sync.dma_start(out=b1[:], in_=b1_d[:, :])
            nc.vector.memset(eps_t[:], LN_EPS)
            if general:
                ones1 = singles.tile([1, 128], BF16)
                nc.vector.memset(ones1[:], 1.0)
                b2row = singles.tile([1, H], BF16)
                nc.sync.dma_start(out=b2row[:], in_=b2_d[:, :])
                w2s_mb2 = singles.tile([1, 1], BF16)
                nc.sync.dma_start(out=w2s_mb2[:], in_=mb2_d[:, :])
                gam4 = singles.tile([128, K_BLK, H], F32)
                bet4 = singles.tile([128, K_BLK, H], F32)
                nc.sync.dma_start(out=gam4[:], in_=gam_d[:, :, :])
                nc.sync.dma_start(out=bet4[:], in_=bet_d[:, :, :])

            for s in range(n_superblocks):
                e0 = s * SB
                idx_tile_shape = [1, SB] if GATHER_MODE == "flat" else [128, K_SB]
                idx_s = sbp.tile(idx_tile_shape, I32, tag="idxs")
                idx_t = sbp.tile(idx_tile_shape, I32, tag="idxd")
                nc.sync.dma_start(out=idx_s[:], in_=si_d[s, :, :])
                nc.sync.dma_start(out=idx_t[:], in_=di_d[s, :, :])

                ef32 = sbp.tile([128, K_SB, H], F32, tag="ef32")
                nc.sync.dma_start(
                    out=ef32[:],
                    in_=ef_d[e0 : e0 + SB, :].rearrange("(p k) c -> p k c", k=K_SB),
                )
                efbf = sbp.tile([128, K_SB, H], BF16, tag="efbf")
                nc.scalar.activation(
                    out=efbf[:], in_=ef32[:], func=mybir.ActivationFunctionType.Copy
                )
                out_t = sbp.tile([128, K_SB, H], F32, tag="outt")

                for b in range(BLKS_PER_SB):
                    j0 = b * K_BLK
                    # ---- gather node rows for this block's 512 edges ----
                    src_g = blk.tile([128, K_BLK, NID], BF16, tag="srcg")
                    dst_g = blk.tile([128, K_BLK, NID], BF16, tag="dstg")
                    for g_out, g_idx in ((src_g, idx_s), (dst_g, idx_t)):
                        if GATHER_MODE == "multi":
                            nc.gpsimd.indirect_dma_start(
                                out=g_out[:],
                                out_offset=None,
                                in_=tbl[:, :],
                                in_offset=bass.IndirectOffsetOnAxis(
                                    ap=g_idx[:, j0 : j0 + K_BLK], axis=0
                                ),
                            )
                        elif GATHER_MODE == "per128":
                            for j in range(K_BLK):
                                nc.gpsimd.indirect_dma_start(
                                    out=g_out[:, j, :],
                                    out_offset=None,
                                    in_=tbl[:, :],
                                    in_offset=bass.IndirectOffsetOnAxis(
                                        ap=g_idx[:, j0 + j : j0 + j + 1], axis=0
                                    ),
                                )
                        elif GATHER_MODE == "flat":
                            nc.gpsimd.indirect_dma_start(
                                out=g_out[:],
                                out_offset=None,
                                in_=tbl[:, :],
                                in_offset=bass.IndirectOffsetOnAxis(
                                    ap=g_idx[:1, b * BLK : (b + 1) * BLK], axis=0
                                ),
                            )
                        else:
                            raise ValueError(GATHER_MODE)

                    # ---- transpose to channel-major [feat, edge] ----
                    srcT_ps = ps_tr.tile([NID, K_BLK, 128], BF16, tag="tr")
                    dstT_ps = ps_tr.tile([NID, K_BLK, 128], BF16, tag="tr")
                    efT_ps = ps_tr.tile([H, K_BLK, 128], BF16, tag="tr")
                    for j in range(K_BLK):
                        nc.tensor.matmul(
                            srcT_ps[:, j, :], src_g[:, j, :], ident[:],
                            is_transpose=True, skip_group_check=True,
                        )
                        nc.tensor.matmul(
                            dstT_ps[:, j, :], dst_g[:, j, :], ident[:],
                            is_transpose=True, skip_group_check=True,
                        )
                        nc.tensor.matmul(
                            efT_ps[:, j, :], efbf[:, j0 + j, :], ident[:],
                            is_transpose=True, skip_group_check=True,
                        )
                    srcT = blk.tile([NID, K_BLK, 128], BF16, tag="srcT")
                    dstT = blk.tile([NID, K_BLK, 128], BF16, tag="dstT")
                    efT = blk.tile([H, K_BLK, 128], BF16, tag="efT")
                    nc.vector.tensor_copy(out=srcT[:], in_=srcT_ps[:])
                    nc.vector.tensor_copy(out=dstT[:], in_=dstT_ps[:])
                    nc.scalar.copy(out=efT[:], in_=efT_ps[:])

                    # ---- h1 = x @ W1 (channel-major, accumulate 3 chunks) ----
                    h1_ps = ps_h1.tile([H, BLK], F32, tag="h1")
                    nc.tensor.matmul(
                        h1_ps[:], w1a[:], srcT[:].rearrange("p k e -> p (k e)"),
                        start=True, stop=False,
                    )
                    nc.tensor.matmul(
                        h1_ps[:], w1b[:], dstT[:].rearrange("p k e -> p (k e)"),
                        start=False, stop=False,
                    )
                    nc.tensor.matmul(
                        h1_ps[:], w1c[:], efT[:].rearrange("p k e -> p (k e)"),
                        start=False, stop=True,
                    )

                    # ---- relu(h1 + b1) -> bf16 (stationary for mm2) ----
                    g1 = blk.tile([H, BLK], BF16, tag="g1")
                    nc.scalar.activation(
                        out=g1[:], in_=h1_ps[:],
                        func=mybir.ActivationFunctionType.Relu,
                        bias=b1[:],
                    )

                    # ---- h = g1.T @ W2 (edge-major) + row-sum -> mean ----
                    h_ps = ps_h.tile([128, K_BLK, H], F32, tag="h")
                    mu_ps = ps_tr.tile([128, K_BLK], F32, tag="tr")
                    for j in range(K_BLK):
                        lhs = g1[:, j * 128 : (j + 1) * 128]
                        nc.tensor.matmul(
                            h_ps[:, j, :], lhs, w2[:],
                            start=True, stop=not general, skip_group_check=True,
                        )
                        nc.tensor.matmul(
                            mu_ps[:, j : j + 1], lhs, w2s[:],
                            start=True, stop=not general, skip_group_check=True,
                        )
                        if general:
                            # += ones.T @ b2  and mean(b2)
                            nc.tensor.matmul(
                                h_ps[:, j, :], ones1[:, :], b2row[:, :],
                                start=False, stop=True, skip_group_check=True,
                            )
                            nc.tensor.matmul(
                                mu_ps[:, j : j + 1], ones1[:, :], w2s_mb2[:, :],
                                start=False, stop=True, skip_group_check=True,
                            )

                    # ---- LN stats: mu from matmul; E[h^2] via ACT square ----
                    mu = blk.tile([128, K_BLK], F32, tag="mu")
                    nc.vector.tensor_copy(out=mu[:], in_=mu_ps[:])
                    eh2 = blk.tile([128, K_BLK], F32, tag="eh2")
                    sq_trash = blk.tile([128, H], BF16, tag="sqt")
                    for j in range(K_BLK):
                        nc.scalar.activation(
                            out=sq_trash[:], in_=h_ps[:, j, :],
                            func=mybir.ActivationFunctionType.Square,
                            scale=float(1.0 / np.sqrt(H)),
                            accum_out=eh2[:, j : j + 1],
                        )
                    mu2 = blk.tile([128, K_BLK], F32, tag="mu2")
                    nc.vector.tensor_mul(out=mu2[:], in0=mu[:], in1=mu[:])
                    var = blk.tile([128, K_BLK], F32, tag="var")
                    nc.vector.tensor_tensor(
                        out=var[:], in0=eh2[:], in1=mu2[:],
                        op=mybir.AluOpType.subtract,
                    )
                    sig = blk.tile([128, K_BLK], F32, tag="sig")
                    nc.scalar.activation(
                        out=sig[:], in_=var[:],
                        func=mybir.ActivationFunctionType.Sqrt,
                        bias=eps_t[:],
                    )
                    rstd = blk.tile([128, K_BLK], F32, tag="rstd")
                    nc.vector.reciprocal(out=rstd[:], in_=sig[:])

                    # ---- normalize + residual ----
                    t = blk.tile([128, K_BLK, H], F32, tag="t")
                    for j in range(K_BLK):
                        nc.vector.tensor_scalar(
                            out=t[:, j, :], in0=h_ps[:, j, :],
                            scalar1=mu[:, j : j + 1], scalar2=rstd[:, j : j + 1],
                            op0=mybir.AluOpType.subtract, op1=mybir.AluOpType.mult,
                        )
                    if general:
                        nc.vector.tensor_mul(out=t[:], in0=t[:], in1=gam4[:])
                        nc.vector.tensor_add(out=t[:], in0=t[:], in1=bet4[:])
                    nc.vector.tensor_tensor(
                        out=out_t[:, j0 : j0 + K_BLK, :], in0=t[:],
                        in1=ef32[:, j0 : j0 + K_BLK, :], op=mybir.AluOpType.add,
                    )

                nc.sync.dma_start(
                    out=out_d[e0 : e0 + SB, :].rearrange("(p k) c -> p k c", k=K_SB),
                    in_=out_t[:],
                )

    nc.finalize()
    return nc


_PROG_CACHE: dict = {}


def _get_program(n_superblocks: int, general: bool):
    key = (n_superblocks, general)
    if key not in _PROG_CACHE:
        _PROG_CACHE[key] = _build_program(n_superblocks, general)
    return _PROG_CACHE[key]


def _prep_inputs(node_invariant, edge_features, edge_index, W1, b1, W2, b2,
                 ln_gamma, ln_beta, n_superblocks_per_core=None):
    """Shard + lay out host inputs. Returns (in_maps, general, e_pad, E)."""
    E = edge_features.shape[0]
    general = not (
        np.all(b2 == 0.0) and np.all(ln_gamma == 1.0) and np.all(ln_beta == 0.0)
    )

    if n_superblocks_per_core is None:
        n_sb = -(-E // (N_CORES * SB))  # ceil
    else:
        n_sb = n_superblocks_per_core
    e_core = n_sb * SB
    e_pad = e_core * N_CORES

    ef = np.ascontiguousarray(edge_features, dtype=np.float32)
    si = np.ascontiguousarray(edge_index[0], dtype=np.int32)
    di = np.ascontiguousarray(edge_index[1], dtype=np.int32)
    if e_pad != E:
        pad = e_pad - E
        ef = np.concatenate([ef, np.zeros((pad, H), np.float32)])
        si = np.concatenate([si, np.zeros(pad, np.int32)])
        di = np.concatenate([di, np.zeros(pad, np.int32)])

    tbl = node_invariant.astype(ml_dtypes.bfloat16)
    w1 = np.asarray(W1, np.float32)
    w1a = np.ascontiguousarray(w1[:NID]).astype(ml_dtypes.bfloat16)
    w1b = np.ascontiguousarray(w1[NID : 2 * NID]).astype(ml_dtypes.bfloat16)
    w1c = np.ascontiguousarray(w1[2 * NID :]).astype(ml_dtypes.bfloat16)
    w2 = np.asarray(W2, np.float32)
    w2bf = w2.astype(ml_dtypes.bfloat16)
    w2s = (w2.sum(axis=1, keepdims=True) / H).astype(ml_dtypes.bfloat16)
    b1c = np.asarray(b1, np.float32).reshape(H, 1)

    in_maps = []
    for c in range(N_CORES):
        lo = c * e_core
        hi = lo + e_core
        def _idx_layout(a):
            a = a.reshape(n_sb, 128, K_SB)
            if GATHER_MODE == "flat":
                a = (
                    a.reshape(n_sb, 128, BLKS_PER_SB, K_BLK)
                    .transpose(0, 2, 1, 3)
                    .reshape(n_sb, 1, SB)
                )
            return np.ascontiguousarray(a)

        m = {
            "tbl": tbl,
            "ef": ef[lo:hi],
            "src_idx": _idx_layout(si[lo:hi]),
            "dst_idx": _idx_layout(di[lo:hi]),
            "w1a": w1a, "w1b": w1b, "w1c": w1c,
            "w2": w2bf, "w2sum": w2s, "b1": b1c,
        }
        if general:
            m["b2row"] = np.asarray(b2, np.float32).reshape(1, H).astype(
                ml_dtypes.bfloat16
            )
            m["mb2"] = np.asarray(
                [[np.asarray(b2, np.float32).mean()]], np.float32
            ).astype(ml_dtypes.bfloat16)
            m["gam4"] = np.broadcast_to(
                np.asarray(ln_gamma, np.float32), (128, K_BLK, H)
            ).copy()
            m["bet4"] = np.broadcast_to(
                np.asarray(ln_beta, np.float32), (128, K_BLK, H)
            ).copy()
        in_maps.append(m)
    return in_maps, general, n_sb, E


def run(inputs: dict, trace: bool = False, n_superblocks_per_core=None, **kw):
    """Run on hardware; returns (output, BassKernelResults)."""
    in_maps, general, n_sb, E = _prep_inputs(
        **inputs, n_superblocks_per_core=n_superblocks_per_core
    )
    nc = _get_program(n_sb, general)
    res = run_bass_kernel_spmd(
        nc, in_maps, core_ids=list(range(N_CORES)), trace=trace, **kw
    )
    out = np.concatenate([r["out"] for r in res.results], axis=0)[:E]
    return np.ascontiguousarray(out, dtype=np.float32), res


def kernel(**inputs) -> np.ndarray:
    out, _ = run(inputs, trace=False)
    return out
